# revision 2
# baseline (speedup 1.0000x reference)
"""Causal MHA on 8 TRN2 cores — v3: bf16 + PSUM rebalance + sw pipeline.

v2 changes (bf16 operands, Pool-engine post-exp mask) plus:
- o_proj and the normalization-broadcast matmuls share the projection
  PSUM pool (they're serialized with projections on the PE queue anyway),
  freeing a bank to give the attention-output accumulators 3 banks, so
  head-pair hp+1 can start accumulating while hp's normalization drains.
- The attention kb loop is software-pipelined: scores for kb+1 issue
  before the attn@v matmuls of kb, giving the ScalarE exp stream a full
  extra kb of lookahead (exp is the inner-loop rate limiter).
- vaug ones-column memsets hoisted out of the rep loop.
"""

import numpy as np
import ml_dtypes

import bass_rust
import concourse.bass as bass
import concourse.mybir as mybir
import concourse.tile as tile
from concourse.bass_utils import run_bass_kernel_spmd

N_CORES = 8
B, S, D = 4, 2048, 1024
H, DH = 16, 64
HC = 512          # projection columns per core (8 heads)
BF16 = mybir.dt.bfloat16
F32 = mybir.dt.float32

_ctr = [0]


def _split_multiwaits(nc):
    """walrus here refuses instructions with >1 wait or >1 update (one
    EVENTS slot per 64B instruction); hoist extras onto adjacent NoOps."""
    n = 0
    for fn in nc.m.functions:
        for blk in fn.blocks:
            insts = blk.instructions
            i = 0
            while i < len(insts):
                inst = insts[i]
                si = getattr(inst, "sync_info", None)
                if si is None:
                    i += 1
                    continue
                waits, updates = list(si.on_wait), list(si.on_update)
                changed = False
                if len(waits) > 1:
                    for w in waits[:-1]:
                        _ctr[0] += 1
                        nop = mybir.InstNoOp(
                            engine=inst.engine, name=f"waitsplit_{_ctr[0]}"
                        )
                        nop.sync_info = bass_rust.SyncInfo(
                            on_wait=[w], on_update=[]
                        )
                        insts.insert(i, nop)
                        i += 1
                    waits = waits[-1:]
                    changed = True
                if len(updates) > 1:
                    for j, u in enumerate(updates[1:]):
                        _ctr[0] += 1
                        nop = mybir.InstNoOp(
                            engine=inst.engine, name=f"updsplit_{_ctr[0]}"
                        )
                        nop.sync_info = bass_rust.SyncInfo(
                            on_wait=[], on_update=[u]
                        )
                        insts.insert(i + 1 + j, nop)
                    updates = updates[:1]
                    changed = True
                if changed:
                    inst.sync_info = bass_rust.SyncInfo(
                        on_wait=waits, on_update=updates
                    )
                    n += 1
                i += 1
    return n


def build_bass(n_rep=1):
    nc = bass.Bass("TRN2", target_bir_lowering=False, debug=False,
                   num_devices=N_CORES)
    xT = nc.dram_tensor("xT", [D, S], BF16, kind="ExternalInput")
    wqT = nc.dram_tensor("wqT", [D, HC], BF16, kind="ExternalInput")
    wkT = nc.dram_tensor("wkT", [D, HC], BF16, kind="ExternalInput")
    wvT = nc.dram_tensor("wvT", [D, HC], BF16, kind="ExternalInput")
    owT = nc.dram_tensor("owT", [HC, D], BF16, kind="ExternalInput")
    # mask01[k, q] = 1 where k <= q else 0 (diagonal 128x128 causal tri)
    maskb = nc.dram_tensor("maskb", [128, 128], BF16, kind="ExternalInput")
    out = nc.dram_tensor("out", [S, D], F32, kind="ExternalOutput")

    ND = D // 128     # 8 d tiles

    with tile.TileContext(nc) as tc:
        with tc.tile_pool(name="perm", bufs=1) as perm, \
             tc.tile_pool(name="wk_", bufs=1) as wpool, \
             tc.tile_pool(name="str", bufs=1) as st, \
             tc.tile_pool(name="pps", bufs=3, space="PSUM") as pps, \
             tc.tile_pool(name="scs", bufs=3, space="PSUM") as scs, \
             tc.tile_pool(name="oas", bufs=2, space="PSUM") as oas:
            mask_sb = perm.tile([128, 128], BF16, tag="mask", name="mask_sb")
            nc.sync.dma_start(out=mask_sb, in_=maskb[:, :])
            ones64 = perm.tile([1, 64], BF16, tag="ones", name="ones64")
            nc.vector.memset(ones64, 1.0)
            kT = [[perm.tile([128, 512], BF16, tag=f"kT{i}_{j}",
                             name=f"kT{i}_{j}") for j in range(4)]
                  for i in range(4)]
            vaug = [perm.tile([128, 8 * 65], BF16, tag=f"va{i}", name=f"va{i}")
                    for i in range(16)]
            wq = wpool.tile([128, 8, HC], BF16, tag="wq", name="wq")
            wk = wpool.tile([128, 8, HC], BF16, tag="wk", name="wk")
            wv = wpool.tile([128, 8, HC], BF16, tag="wv", name="wv")
            ow = wpool.tile([128, 4, D], BF16, tag="ow", name="ow")

            for i in range(16):
                nc.gpsimd.memset(vaug[i][:, :].rearrange(
                    "p (h c) -> p h c", h=8)[:, :, 64:65], 1.0)

            for _rep in range(n_rep):
                nc.sync.dma_start(
                    out=wk, in_=wkT.rearrange("(d p) c -> p d c", p=128))

                for sc in range(4):      # s/q chunk of 512 (qb == sc)
                    q0 = 512 * sc
                    # ---- projections for this chunk ----
                    xc = st.tile([128, 8, 512], BF16, tag="xc",
                                 name="xc", bufs=2)
                    nc.sync.dma_start(
                        out=xc,
                        in_=xT.rearrange("(d p) s -> p d s",
                                         p=128)[:, :, q0:q0 + 512])
                    if sc == 0:
                        nc.sync.dma_start(
                            out=wq,
                            in_=wqT.rearrange("(d p) c -> p d c", p=128))
                        nc.sync.dma_start(
                            out=wv,
                            in_=wvT.rearrange("(d p) c -> p d c", p=128))
                        nc.sync.dma_start(
                            out=ow,
                            in_=owT.rearrange("(v p) m -> p v m", p=128))
                    qTc = [st.tile([128, 512], BF16, tag=f"qc{i}",
                                   name=f"qc{i}", bufs=2) for i in range(4)]
                    # Accumulating matmuls into the SAME psum bank back-to-back
                    # run at half rate (read-modify-write port conflict);
                    # interleave pairs of accumulation groups across banks.
                    for w, dst in ((wk, kT), (wq, qTc)):
                        for cbp in range(2):
                            cba, cbb = 2 * cbp, 2 * cbp + 1
                            psa = pps.tile([128, 512], F32, tag="pp",
                                           name="psa")
                            psb = pps.tile([128, 512], F32, tag="pp",
                                           name="psb")
                            for d in range(ND):
                                nc.tensor.matmul(
                                    psa, w[:, d, 128*cba:128*(cba+1)],
                                    xc[:, d, :],
                                    start=(d == 0), stop=(d == ND - 1))
                                nc.tensor.matmul(
                                    psb, w[:, d, 128*cbb:128*(cbb+1)],
                                    xc[:, d, :],
                                    start=(d == 0), stop=(d == ND - 1))
                            for cb, ps in ((cba, psa), (cbb, psb)):
                                if dst is kT:
                                    nc.vector.tensor_copy(
                                        out=dst[cb][sc], in_=ps)
                                else:
                                    nc.vector.tensor_copy(out=dst[cb], in_=ps)
                    for sp in range(2):
                        ssa, ssb_ = 2 * sp, 2 * sp + 1
                        psa = pps.tile([128, 512], F32, tag="pp", name="psa")
                        psb = pps.tile([128, 512], F32, tag="pp", name="psb")
                        for d in range(ND):
                            nc.tensor.matmul(
                                psa, xc[:, d, 128*ssa:128*(ssa+1)],
                                wv[:, d, :],
                                start=(d == 0), stop=(d == ND - 1))
                            nc.tensor.matmul(
                                psb, xc[:, d, 128*ssb_:128*(ssb_+1)],
                                wv[:, d, :],
                                start=(d == 0), stop=(d == ND - 1))
                        for ss, ps in ((ssa, psa), (ssb_, psb)):
                            sb = 4 * sc + ss
                            nc.vector.tensor_copy(
                                out=vaug[sb][:, :].rearrange(
                                    "p (h c) -> p h c", h=8)[:, :, 0:64],
                                in_=ps[:, :].rearrange("p (h c) -> p h c",
                                                       h=8))

                    # ---- attention for q block sc ----
                    nkb = 4 * (sc + 1)
                    aoTc = [st.tile([128, 512], BF16, tag=f"ao{i}",
                                    name=f"ao{i}", bufs=2) for i in range(4)]
                    for hp in range(4):
                        oa = [None, None]
                        for hi in range(2):
                            oa[hi] = oas.tile([65, 512], F32, tag="oa",
                                              name="oa")

                        def issue_oa(pend_kb, pend_exs):
                            for hi, ex, q_ in pend_exs:
                                h = 2 * hp + hi
                                nc.tensor.matmul(
                                    oa[hi][:, q_:512],
                                    vaug[pend_kb][:, 65*h:65*h + 65],
                                    ex[:, q_:512],
                                    start=(pend_kb == 0),
                                    stop=(pend_kb == nkb - 1))

                        pend = None
                        for kb in range(nkb):
                            di = kb - 4 * sc
                            qlo = 128 * di if di > 0 else 0
                            sc_ps = {}
                            for hi in range(2):
                                prow = slice(64 * hi, 64 * hi + 64)
                                sps = scs.tile([128, 512], F32, tag="sc",
                                               name="sps")
                                nc.tensor.matmul(
                                    sps[:, qlo:512],
                                    kT[hp][kb // 4][prow,
                                                    128*(kb % 4):
                                                    128*(kb % 4 + 1)],
                                    qTc[hp][prow, qlo:512],
                                    start=True, stop=True)
                                sc_ps[hi] = sps
                            exs = []
                            for hi in range(2):
                                sps = sc_ps[hi]
                                ex = st.tile([128, 512], BF16, tag="ex",
                                             name="ex", bufs=5)
                                nc.scalar.activation(
                                    out=ex[:, qlo:512], in_=sps[:, qlo:512],
                                    func=mybir.ActivationFunctionType.Exp,
                                    scale=0.125)
                                if di >= 0:
                                    # zero the k>q triangle of the diagonal
                                    # block (post-exp 0/1 mask, Pool engine)
                                    nc.gpsimd.tensor_mul(
                                        ex[:, qlo:qlo + 128],
                                        ex[:, qlo:qlo + 128], mask_sb)
                                exs.append((hi, ex, qlo))
                            # attn@v for the previous kb (one-kb lookahead
                            # keeps the exp stream off the PE critical path)
                            if pend is not None:
                                issue_oa(*pend)
                            pend = (kb, exs)
                        issue_oa(*pend)
                        for hi in range(2):
                            prow = slice(64 * hi, 64 * hi + 64)
                            rc = st.tile([1, 512], BF16, tag="rc", name="rc",
                                         bufs=1)
                            with nc.allow_low_precision(
                                    reason="bf16 recip feeds bcast matmul"):
                                nc.vector.reciprocal(
                                    out=rc, in_=oa[hi][64:65, :])
                            bcps = pps.tile([64, 512], F32, tag="pp",
                                            name="bcps")
                            nc.tensor.matmul(bcps, ones64, rc,
                                             start=True, stop=True)
                            bc = st.tile([64, 512], F32, tag="bc", name="bc",
                                         bufs=1)
                            nc.vector.tensor_copy(out=bc, in_=bcps)
                            nc.vector.tensor_mul(
                                aoTc[hp][prow, :], oa[hi][0:64, :], bc)

                    # ---- o_proj for this chunk ----
                    for ssb in range(4):
                        ot = st.tile([128, 1024], F32, tag="ot",
                                     name="ot", bufs=2)
                        psa = pps.tile([128, 512], F32, tag="pp", name="psa")
                        psb = pps.tile([128, 512], F32, tag="pp", name="psb")
                        for v in range(4):
                            nc.tensor.matmul(
                                psa,
                                aoTc[v][:, 128*ssb:128*(ssb+1)],
                                ow[:, v, 0:512],
                                start=(v == 0), stop=(v == 3))
                            nc.tensor.matmul(
                                psb,
                                aoTc[v][:, 128*ssb:128*(ssb+1)],
                                ow[:, v, 512:1024],
                                start=(v == 0), stop=(v == 3))
                        nc.scalar.activation(
                            out=ot[:, 0:512], in_=psa,
                            func=mybir.ActivationFunctionType.Copy)
                        nc.scalar.activation(
                            out=ot[:, 512:1024], in_=psb,
                            func=mybir.ActivationFunctionType.Copy)
                        nc.sync.dma_start(
                            out=out[q0 + 128*ssb:q0 + 128*(ssb+1), :],
                            in_=ot)

    _split_multiwaits(nc)
    return nc


_CACHE = {}


def _get_nc():
    if "nc" not in _CACHE:
        _CACHE["nc"] = build_bass()
    return _CACHE["nc"]


def _prepare_inputs(x, q_w, k_w, v_w, o_w):
    bf = ml_dtypes.bfloat16
    mask = np.where(
        np.arange(128)[:, None] <= np.arange(128)[None, :], 1.0, 0.0
    ).astype(bf)
    in_maps = []
    xTs = [np.ascontiguousarray(x[b].T.astype(bf)) for b in range(B)]
    for c in range(N_CORES):
        b, hh = c // 2, c % 2
        cols = slice(HC * hh, HC * (hh + 1))
        in_maps.append({
            "xT": xTs[b],
            "wqT": np.ascontiguousarray(q_w.T[:, cols].astype(bf)),
            "wkT": np.ascontiguousarray(k_w.T[:, cols].astype(bf)),
            "wvT": np.ascontiguousarray(v_w.T[:, cols].astype(bf)),
            "owT": np.ascontiguousarray(o_w.T[cols, :].astype(bf)),
            "maskb": mask,
        })
    return in_maps


def kernel(x, q_proj_weight, k_proj_weight, v_proj_weight, o_proj_weight):
    x = np.asarray(x, dtype=np.float32)
    q_w = np.asarray(q_proj_weight, dtype=np.float32)
    k_w = np.asarray(k_proj_weight, dtype=np.float32)
    v_w = np.asarray(v_proj_weight, dtype=np.float32)
    o_w = np.asarray(o_proj_weight, dtype=np.float32)

    nc = _get_nc()
    in_maps = _prepare_inputs(x, q_w, k_w, v_w, o_w)
    res = run_bass_kernel_spmd(nc, in_maps, core_ids=list(range(N_CORES)))
    outp = np.empty((B, S, D), dtype=np.float32)
    for b in range(B):
        outp[b] = res.results[2 * b]["out"] + res.results[2 * b + 1]["out"]
    return outp
